# revision 16
# baseline (speedup 1.0000x reference)
"""Trainium2 Bass kernel: multi-scale masked average-pool descriptors.

Computes, per batch element b and scribble i:
    d_l[b,i,c] = mean over {pixels where resize(scribble)[b,i,y,x] > 0.5} of feat_l[b,c,y,x]
    out[b,i,c] = (d_0 + d_1 + d_2) / 3

Strategy (v2: DMA-packet + PE-stride optimized):
  * jax.image.resize(bilinear, antialias=False) at scales 4/8/16 reduces to an
    exact 2x2 average at stride k with offset o (k,o) = (4,1)/(8,3)/(16,7):
    mask == ((a+c)+(b+d)) > 2.0 bit-exactly in fp32 (computed on DVE).
  * Feature maps are DMA'd in their NATIVE [C, h*w] layout (contiguous 4-16KB
    descriptors instead of the 128-256B x-runs a [y, C, x] layout forces)
    through the gpsimd SWDGE, which casts fp32->bf16 inline.  The [c, s]
    tiles are then transposed on-chip by the DMA xbar transpose (16-bit only,
    hence the cast) into [s(part), c] pixel-group tiles.
  * The masked sum is ssum[i,c] = sum_s maskT[s,i]*fT[s,c]: one bf16 matmul
    per 128-pixel raster group (K=128, M=16, N=256, contiguous rhs) -- 168
    matmuls instead of 224 strided fp32 ones (which cost 390-480ns each).
  * Masks are built in per-level layouts such that a single 2D xbar transpose
    of each mask slice yields the matmul lhsT [128px, 16img] directly.  The
    xbar raster fold makes image order come out permuted for levels 1/2; the
    host un-permutes (free).
  * cnt[i] (mask popcount) = reduce_sum over the transposed mask + a ones
    matmul; bf16 masks are exact 0/1 and PSUM accumulates fp32, so cnt is
    exact and the masks match the reference bit-exactly.
  * Scribbles ride the two HWDGE rings (sync/scalar) as 4KB merged row-pair
    descriptors; features ride the SWDGE ring; xbar transposes interleave on
    the HWDGE rings.
  * The empty-mask fallback is handled on the host (P(empty) ~ 2^-1024).

Sharding: pure data-parallel over batch B=8 across the 8 NeuronCores.
"""

import numpy as np

_B = 8
_I = 16
_C = 256

# level: (h, k, off)
_LEVELS = {0: (128, 4, 1), 1: (64, 8, 3), 2: (32, 16, 7)}

# xbar-induced image permutation per level: matmul M position of image i
_IMG2POS = {
    0: list(range(16)),
    1: [(i % 8) * 2 + i // 8 for i in range(16)],  # i=8*ihi+ilo -> b=2*ilo+ihi
    2: [(i % 4) * 4 + i // 4 for i in range(16)],  # i=4*ihi+ilo -> b=4*ilo+ihi
}


def _build_nc():
    import concourse.bacc as bacc
    import concourse.tile as tile
    from concourse import mybir

    f32 = mybir.dt.float32
    bf16 = mybir.dt.bfloat16
    gt = mybir.AluOpType.is_gt
    X = mybir.AxisListType.X

    nc = bacc.Bacc("TRN2", target_bir_lowering=False, debug=False)

    feats = {
        0: nc.dram_tensor("feat0", [_C, 128, 128], f32, kind="ExternalInput"),
        1: nc.dram_tensor("feat1", [_C, 64, 64], f32, kind="ExternalInput"),
        2: nc.dram_tensor("feat2", [_C, 32, 32], f32, kind="ExternalInput"),
    }
    scr = nc.dram_tensor("scribbles", [_I, 512, 512], f32, kind="ExternalInput")
    out_d = nc.dram_tensor("out", [_I, 3 * (_C + 1)], f32, kind="ExternalOutput")

    with tile.TileContext(nc) as tc:
        with (
            tc.tile_pool(name="singles", bufs=1) as singles,
            tc.tile_pool(name="scrib", bufs=2) as scrib,
            tc.tile_pool(name="tmp", bufs=1) as tmp,
            tc.tile_pool(name="sg", bufs=3) as sgp,
            tc.tile_pool(name="tp", bufs=2) as tpp,
            tc.tile_pool(name="psum", bufs=3, space="PSUM") as psum,
        ):
            ones = singles.tile([128, 1], f32, tag="ones")
            nc.vector.memset(ones[:], 1.0)
            stag = singles.tile([_I, 3 * (_C + 1)], f32, tag="stag")

            # mask tiles: msk* are the DVE-side layouts, mT* the xbar outputs
            # (matmul lhsT [128px, 16img] at M position b = _IMG2POS[l][i]).
            # Partition folds (q = scribble-tile partition); all xbar outs are
            # dense 2D after free-dim merging (the HW xbar needs that):
            #   L0: msk0[y, i, x];                 mT0[x, i, y]
            #   L1: msk1[q=32*ihi+g, ilo, ylo, x]; mT1[p, b=2*ilo+ihi, g]
            #   L2: msk2[q=8*ihi+g, ilo, yq, x];   mT2[p, b=4*ilo+ihi, g]
            msk0 = singles.tile([128, _I, 128], bf16, tag="msk0")
            mT0 = singles.tile([128, _I, 128], bf16, tag="mT0")
            msk1 = singles.tile([64, 8, 2, 64], bf16, tag="msk1")
            mT1 = singles.tile([128, _I, 32], bf16, tag="mT1")
            msk2 = singles.tile([32, 4, 4, 32], bf16, tag="msk2")
            mT2 = singles.tile([128, _I, 8], bf16, tag="mT2")


            def flat2(ap):
                """Collapse an AP's free dims into one (explicit 2D view)."""
                n = len(ap.shape)
                dims = " ".join(f"d{j}" for j in range(1, n))
                return ap.rearrange(f"p {dims} -> p ({dims})")

            # =============== level 2 ===============
            for t in range(4):
                st = scrib.tile([32, 4, 2, 512], f32, tag="st2")
                for yq in range(4):
                    nc.scalar.dma_start(
                        out=st[:, yq, :, :],
                        in_=scr.rearrange(
                            "(ihi ilo) (g r) x -> ilo ihi g r x", ihi=4, r=64
                        )[t, :, :, 16 * yq + 7 : 16 * yq + 9, :],
                    )
                v = tmp.tile([32, 4, 512], f32, tag="v2")
                nc.vector.tensor_add(v[:], st[:, :, 0, :], st[:, :, 1, :])
                vk = v[:].rearrange("p q (x k) -> p q x k", k=16)
                sr = tmp.tile([32, 4, 32], f32, tag="sr2")
                nc.vector.tensor_add(sr[:], vk[:, :, :, 7], vk[:, :, :, 8])
                nc.vector.tensor_scalar(
                    out=msk2[:, t, :, :], in0=sr[:], scalar1=2.0, scalar2=None,
                    op0=gt,
                )
            for ilo in range(4):
                nc.sync.dma_start(
                    out=mT2[:, 4 * ilo : 4 * ilo + 4, :],
                    in_=flat2(msk2[:, ilo, :, :]),
                    transpose=True,
                )

            acc2 = psum.tile([_I, _C], f32, tag="acc")
            T2 = tpp.tile([128, 8, 2, 128], bf16, tag="T2")
            for H in range(2):
                sg = sgp.tile([128, 1024], bf16, tag="sg1k")
                nc.gpsimd.dma_start(
                    out=sg[:],
                    in_=feats[2].rearrange("c y x -> c (y x)")[
                        128 * H : 128 * (H + 1), :
                    ],
                )
                nc.sync.dma_start(out=T2[:, :, H, :], in_=sg[:], transpose=True)
            for g in range(8):
                nc.tensor.matmul(
                    acc2[:], mT2[:, :, g], T2[:, g, :, :],
                    start=(g == 0), stop=(g == 7),
                )
            r2 = singles.tile([128, _I], f32, tag="r2")
            nc.vector.reduce_sum(out=r2[:], in_=mT2[:], axis=X)
            cnt2 = psum.tile([_I, 1], f32, tag="cnt")
            nc.tensor.matmul(cnt2[:], r2[:], ones[:], start=True, stop=True)
            base = 2 * (_C + 1)
            nc.vector.tensor_copy(stag[:, base : base + _C], acc2[:])
            nc.vector.tensor_copy(stag[:, base + _C : base + _C + 1], cnt2[:])

            # =============== level 1 ===============
            for t in range(8):
                st = scrib.tile([64, 2, 2, 512], f32, tag="st1")
                for ylo in range(2):
                    nc.scalar.dma_start(
                        out=st[:, ylo, :, :],
                        in_=scr.rearrange(
                            "(ihi ilo) (g r) x -> ilo ihi g r x", ihi=2, r=16
                        )[t, :, :, 8 * ylo + 3 : 8 * ylo + 5, :],
                    )
                v = tmp.tile([64, 2, 512], f32, tag="v1")
                nc.vector.tensor_add(v[:], st[:, :, 0, :], st[:, :, 1, :])
                vk = v[:].rearrange("p q (x k) -> p q x k", k=8)
                sr = tmp.tile([64, 2, 64], f32, tag="sr1")
                nc.vector.tensor_add(sr[:], vk[:, :, :, 3], vk[:, :, :, 4])
                nc.vector.tensor_scalar(
                    out=msk1[:, t, :, :], in0=sr[:], scalar1=2.0, scalar2=None,
                    op0=gt,
                )
            for ilo in range(8):
                nc.sync.dma_start(
                    out=mT1[:, 2 * ilo : 2 * ilo + 2, :],
                    in_=flat2(msk1[:, ilo, :, :]),
                    transpose=True,
                )

            acc1 = psum.tile([_I, _C], f32, tag="acc")
            T1 = tpp.tile([128, 32, 2, 128], bf16, tag="Tbig")
            for H in range(2):
                sg = sgp.tile([128, 4096], bf16, tag="sg4k")
                nc.gpsimd.dma_start(
                    out=sg[:],
                    in_=feats[1].rearrange("c y x -> c (y x)")[
                        128 * H : 128 * (H + 1), :
                    ],
                )
                nc.sync.dma_start(out=T1[:, :, H, :], in_=sg[:], transpose=True)
            for g in range(32):
                nc.tensor.matmul(
                    acc1[:], mT1[:, :, g], T1[:, g, :, :],
                    start=(g == 0), stop=(g == 31),
                )
            r1 = singles.tile([128, _I], f32, tag="r1")
            nc.vector.reduce_sum(out=r1[:], in_=mT1[:], axis=X)
            cnt1 = psum.tile([_I, 1], f32, tag="cnt")
            nc.tensor.matmul(cnt1[:], r1[:], ones[:], start=True, stop=True)
            base = 1 * (_C + 1)
            nc.vector.tensor_copy(stag[:, base : base + _C], acc1[:])
            nc.vector.tensor_copy(stag[:, base + _C : base + _C + 1], cnt1[:])

            # =============== level 0 ===============
            for tpair in range(8):
                i0 = tpair * 2
                st = scrib.tile([128, 2, 1024], f32, tag="st0")
                nc.scalar.dma_start(
                    out=st[:],
                    in_=scr[i0 : i0 + 2]
                    .rearrange("i (y k) x -> y i k x", k=4)[:, :, 1:3, :]
                    .rearrange("y i k x -> y i (k x)"),
                )
                for il in range(2):
                    v = tmp.tile([128, 512], f32, tag="v0")
                    nc.vector.tensor_add(
                        v[:], st[:, il, 0:512], st[:, il, 512:1024]
                    )
                    vk = v[:].rearrange("p (x k) -> p x k", k=4)
                    sr = tmp.tile([128, 128], f32, tag="sr0")
                    nc.vector.tensor_add(sr[:], vk[:, :, 1], vk[:, :, 2])
                    nc.vector.tensor_scalar(
                        out=msk0[:, i0 + il, :], in0=sr[:], scalar1=2.0,
                        scalar2=None, op0=gt,
                    )
            for i in range(_I):
                nc.sync.dma_start(
                    out=mT0[:, i, :], in_=msk0[:, i, :], transpose=True
                )

            acc0 = psum.tile([_I, _C], f32, tag="acc")
            for chunk in range(4):
                T0 = tpp.tile([128, 32, 2, 128], bf16, tag="Tbig")
                for H in range(2):
                    sg = sgp.tile([128, 4096], bf16, tag="sg4k")
                    nc.gpsimd.dma_start(
                        out=sg[:],
                        in_=feats[0].rearrange("c y x -> c (y x)")[
                            128 * H : 128 * (H + 1),
                            4096 * chunk : 4096 * (chunk + 1),
                        ],
                    )
                    nc.sync.dma_start(out=T0[:, :, H, :], in_=sg[:], transpose=True)
                for g in range(32):
                    xi = chunk * 32 + g
                    nc.tensor.matmul(
                        acc0[:], mT0[:, :, xi], T0[:, g, :, :],
                        start=(xi == 0), stop=(xi == 127),
                    )
            r0 = singles.tile([128, _I], f32, tag="r0")
            nc.vector.reduce_sum(out=r0[:], in_=mT0[:], axis=X)
            cnt0 = psum.tile([_I, 1], f32, tag="cnt")
            nc.tensor.matmul(cnt0[:], r0[:], ones[:], start=True, stop=True)
            nc.vector.tensor_copy(stag[:, 0:_C], acc0[:])
            nc.vector.tensor_copy(stag[:, _C : _C + 1], cnt0[:])

            nc.gpsimd.dma_start(out=out_d[:], in_=stag[:])

    nc.compile()
    return nc


def _host_fallback(scr_bi, fmap_b, h, k, off):
    """Feature at argmax of the soft mask; only used when a mask is empty."""
    V = scr_bi[off::k, :][:h].astype(np.float32) + scr_bi[off + 1 :: k, :][:h]
    sr4 = V[:, off::k][:, :h] + V[:, off + 1 :: k][:, :h]
    idx = int(np.argmax(np.float32(0.25) * sr4))
    y, x = divmod(idx, h)
    return fmap_b[:, y, x]


def kernel(feat0, feat1, feat2, scribbles):
    import sys

    for p in ("/opt/trn_rl_repo", "/opt/pypackages"):
        if p not in sys.path:
            sys.path.append(p)
    from concourse.bass_utils import run_bass_kernel_spmd

    feat0 = np.asarray(feat0, dtype=np.float32)
    feat1 = np.asarray(feat1, dtype=np.float32)
    feat2 = np.asarray(feat2, dtype=np.float32)
    scribbles = np.asarray(scribbles, dtype=np.float32)

    nc = _build_nc()
    in_maps = [
        {
            "feat0": np.ascontiguousarray(feat0[b]),
            "feat1": np.ascontiguousarray(feat1[b]),
            "feat2": np.ascontiguousarray(feat2[b]),
            "scribbles": np.ascontiguousarray(scribbles[b]),
        }
        for b in range(_B)
    ]
    res = run_bass_kernel_spmd(nc, in_maps, core_ids=list(range(_B)))
    raw = np.stack([res.results[b]["out"] for b in range(_B)])  # [B, I, 3*257]
    raw = raw.reshape(_B, _I, 3, _C + 1)
    # un-permute the xbar-induced image order: row _IMG2POS[l][i] holds image i
    for li in (1, 2):
        raw[:, :, li, :] = raw[:, _IMG2POS[li], li, :]
    ssum = raw[..., :_C].astype(np.float32)  # [B, I, 3, C]
    cnt = raw[..., _C].astype(np.float32)  # [B, I, 3]

    mean = ssum / np.maximum(cnt, np.float32(1.0))[..., None]

    if (cnt == 0).any():  # never for non-degenerate inputs
        fm = [feat0, feat1, feat2]
        for b, i, li in zip(*np.nonzero(cnt == 0)):
            h, k, off = _LEVELS[li]
            mean[b, i, li] = _host_fallback(scribbles[b, i], fm[li][b], h, k, off)

    out = (mean[:, :, 0] + mean[:, :, 1] + mean[:, :, 2]) / np.float32(3.0)
    return out.astype(np.float32)


# revision 17
# speedup vs baseline: 1.4221x; 1.4221x over previous
"""Trainium2 Bass kernel: multi-scale masked average-pool descriptors.

Computes, per batch element b and scribble i:
    d_l[b,i,c] = mean over {pixels where resize(scribble)[b,i,y,x] > 0.5} of feat_l[b,c,y,x]
    out[b,i,c] = (d_0 + d_1 + d_2) / 3

Strategy (v4 -- all-measured design):
  * jax.image.resize(bilinear, antialias=False) at scales 4/8/16 reduces to an
    exact 2x2 average at stride k with offset o (k,o) = (4,1)/(8,3)/(16,7):
    mask == ((a+c)+(b+d)) > 2.0 bit-exactly in fp32 (computed on DVE).
    Scribbles ride the gpsimd SWDGE queue as merged 4KB row-pair descriptors.
  * Feature maps are DMA'd with FULL-ROW descriptors ([y, c-group, x] tiles,
    one 512/256/128B descriptor per (c,y) row) on the two HWDGE rings -- the
    DMA descriptor walk performs the [c,y,x] -> [y,...] partition transpose
    for free (~233 GB/s measured; the xbar and PE transpose alternatives
    measured slower and/or serialize against all other DMA).
  * Engine copies re-pack each c-group [y, 32c, w] fp32 into assembled
    [y, x, 256c] bf16 tiles (cast during copy), so every matmul rhs is a
    contiguous [h, 256] bf16 slice.
  * ssum[i,:] accumulates as one matmul per pixel column x: lhsT =
    mask[:, :, x] [h, 16] bf16, rhs = f[:, x, :] [h, 256] bf16 -- measured
    251ns per LDWEIGHTS+MATMUL pair (strided rhs would be 779ns).
  * cnt[i] = reduce_sum over the mask + a ones-matmul; bf16 masks are exact
    0/1 and PSUM accumulates fp32, so cnt is exact and masks match the
    reference bit-exactly.  bf16 features give rel err ~2e-3 (gate: 2e-2).
  * The empty-mask fallback is handled on the host (P(empty) ~ 2^-1024).

Sharding: pure data-parallel over batch B=8 across the 8 NeuronCores.
"""

import numpy as np

_B = 8
_I = 16
_C = 256

# level: (h, k, off)
_LEVELS = {0: (128, 4, 1), 1: (64, 8, 3), 2: (32, 16, 7)}


def _build_nc():
    import concourse.bacc as bacc
    import concourse.tile as tile
    from concourse import mybir

    f32 = mybir.dt.float32
    bf16 = mybir.dt.bfloat16
    gt = mybir.AluOpType.is_gt
    X = mybir.AxisListType.X

    nc = bacc.Bacc("TRN2", target_bir_lowering=False, debug=False)

    feats = {
        0: nc.dram_tensor("feat0", [_C, 128, 128], f32, kind="ExternalInput"),
        1: nc.dram_tensor("feat1", [_C, 64, 64], f32, kind="ExternalInput"),
        2: nc.dram_tensor("feat2", [_C, 32, 32], f32, kind="ExternalInput"),
    }
    scr = nc.dram_tensor("scribbles", [_I, 512, 512], f32, kind="ExternalInput")
    out_d = nc.dram_tensor("out", [_I, 3 * (_C + 1)], f32, kind="ExternalOutput")

    with tile.TileContext(nc) as tc:
        with (
            tc.tile_pool(name="singles", bufs=1) as singles,
            tc.tile_pool(name="scrib", bufs=3) as scrib,
            tc.tile_pool(name="tmp", bufs=2) as tmp,
            tc.tile_pool(name="fR", bufs=2) as fR,
            tc.tile_pool(name="psum", bufs=3, space="PSUM") as psum,
        ):
            ones = singles.tile([128, 1], f32, tag="ones")
            nc.vector.memset(ones[:], 1.0)
            stag = singles.tile([_I, 3 * (_C + 1)], f32, tag="stag")

            # masks, y-on-partitions (natural resize layout): msk_l[y, i, x]
            msk0 = singles.tile([128, _I, 128], bf16, tag="msk0")
            msk1 = singles.tile([64, _I, 64], bf16, tag="msk1")
            msk2 = singles.tile([32, _I, 32], bf16, tag="msk2")
            msk = {0: msk0, 1: msk1, 2: msk2}
            # assembled feature tiles [y, x, c] bf16
            sgT0 = singles.tile([128, 128, _C], bf16, tag="sgT0")
            sgT1 = singles.tile([64, 64, _C], bf16, tag="sgT1")
            sgT2 = singles.tile([32, 32, _C], bf16, tag="sgT2")
            sgT = {0: sgT0, 1: sgT1, 2: sgT2}

            # ---- scribbles (gpsimd SWDGE; 4KB merged row-pair descriptors)
            # L0 first (it gates the longest matmul chain), pairs of images.
            for tpair in range(8):
                i0 = tpair * 2
                st = scrib.tile([128, 2, 1024], f32, tag="st0")
                nc.gpsimd.dma_start(
                    out=st[:],
                    in_=scr[i0 : i0 + 2]
                    .rearrange("i (y k) x -> y i k x", k=4)[:, :, 1:3, :]
                    .rearrange("y i k x -> y i (k x)"),
                )
                for il in range(2):
                    v = tmp.tile([128, 512], f32, tag="v")
                    nc.vector.tensor_add(
                        v[:], st[:, il, 0:512], st[:, il, 512:1024]
                    )
                    vk = v[:].rearrange("p (x k) -> p x k", k=4)
                    sr = tmp.tile([128, 128], f32, tag="sr")
                    nc.vector.tensor_add(sr[:], vk[:, :, 1], vk[:, :, 2])
                    nc.vector.tensor_scalar(
                        out=msk0[:, i0 + il, :], in0=sr[:], scalar1=2.0,
                        scalar2=None, op0=gt,
                    )
            for li, rr in ((1, 8), (2, 16)):
                h, k, off = _LEVELS[li]
                for i in range(_I):
                    st = scrib.tile([h, 2, 512], f32, tag=f"st{li}")
                    nc.gpsimd.dma_start(
                        out=st[:],
                        in_=scr[i].rearrange("(y r) x -> y r x", r=rr)[
                            :, off : off + 2, :
                        ],
                    )
                    v = tmp.tile([h, 512], f32, tag="v")
                    nc.vector.tensor_add(v[:], st[:, 0, :], st[:, 1, :])
                    vk = v[:].rearrange("p (x k) -> p x k", k=k)
                    sr = tmp.tile([h, h], f32, tag="sr")
                    nc.vector.tensor_add(sr[:], vk[:, :, off], vk[:, :, off + 1])
                    nc.vector.tensor_scalar(
                        out=msk[li][:, i, :], in0=sr[:], scalar1=2.0,
                        scalar2=None, op0=gt,
                    )

            # ---- features: full-row c-group loads + cast-assembly copies
            ceng = [nc.vector, nc.gpsimd]
            for li in (0, 1, 2):
                h = _LEVELS[li][0]
                for g in range(8):
                    sg = fR.tile([h, 32, h], f32, tag="sgR")
                    deng = nc.sync if g % 2 == 0 else nc.scalar
                    deng.dma_start(
                        out=sg[:],
                        in_=feats[li][32 * g : 32 * (g + 1)].rearrange(
                            "c y x -> y c x"
                        ),
                    )
                    ceng[g % 2].tensor_copy(
                        sgT[li][:, :, 32 * g : 32 * (g + 1)],
                        sg[:].rearrange("p c x -> p x c"),
                    )

            # ---- matmuls + cnt + staging, level order 0, 1, 2
            for li in (0, 1, 2):
                h = _LEVELS[li][0]
                acc = psum.tile([_I, _C], f32, tag="acc")
                for x in range(h):
                    nc.tensor.matmul(
                        acc[:], msk[li][:, :, x], sgT[li][:, x, :],
                        start=(x == 0), stop=(x == h - 1),
                    )
                r = singles.tile([h, _I], f32, tag=f"r{li}")
                nc.vector.reduce_sum(out=r[:], in_=msk[li][:], axis=X)
                cnt = psum.tile([_I, 1], f32, tag="cnt")
                nc.tensor.matmul(cnt[:], r[:], ones[:h, :], start=True, stop=True)
                base = li * (_C + 1)
                nc.vector.tensor_copy(stag[:, base : base + _C], acc[:])
                nc.vector.tensor_copy(stag[:, base + _C : base + _C + 1], cnt[:])

            nc.gpsimd.dma_start(out=out_d[:], in_=stag[:])

    nc.compile()
    return nc


def _host_fallback(scr_bi, fmap_b, h, k, off):
    """Feature at argmax of the soft mask; only used when a mask is empty."""
    V = scr_bi[off::k, :][:h].astype(np.float32) + scr_bi[off + 1 :: k, :][:h]
    sr4 = V[:, off::k][:, :h] + V[:, off + 1 :: k][:, :h]
    idx = int(np.argmax(np.float32(0.25) * sr4))
    y, x = divmod(idx, h)
    return fmap_b[:, y, x]


def kernel(feat0, feat1, feat2, scribbles):
    import sys

    for p in ("/opt/trn_rl_repo", "/opt/pypackages"):
        if p not in sys.path:
            sys.path.append(p)
    from concourse.bass_utils import run_bass_kernel_spmd

    feat0 = np.asarray(feat0, dtype=np.float32)
    feat1 = np.asarray(feat1, dtype=np.float32)
    feat2 = np.asarray(feat2, dtype=np.float32)
    scribbles = np.asarray(scribbles, dtype=np.float32)

    nc = _build_nc()
    in_maps = [
        {
            "feat0": np.ascontiguousarray(feat0[b]),
            "feat1": np.ascontiguousarray(feat1[b]),
            "feat2": np.ascontiguousarray(feat2[b]),
            "scribbles": np.ascontiguousarray(scribbles[b]),
        }
        for b in range(_B)
    ]
    res = run_bass_kernel_spmd(nc, in_maps, core_ids=list(range(_B)))
    raw = np.stack([res.results[b]["out"] for b in range(_B)])  # [B, I, 3*257]
    raw = raw.reshape(_B, _I, 3, _C + 1)
    ssum = raw[..., :_C].astype(np.float32)  # [B, I, 3, C]
    cnt = raw[..., _C].astype(np.float32)  # [B, I, 3]

    mean = ssum / np.maximum(cnt, np.float32(1.0))[..., None]

    if (cnt == 0).any():  # never for non-degenerate inputs
        fm = [feat0, feat1, feat2]
        for b, i, li in zip(*np.nonzero(cnt == 0)):
            h, k, off = _LEVELS[li]
            mean[b, i, li] = _host_fallback(scribbles[b, i], fm[li][b], h, k, off)

    out = (mean[:, :, 0] + mean[:, :, 1] + mean[:, :, 2]) / np.float32(3.0)
    return out.astype(np.float32)


# revision 22
# speedup vs baseline: 1.7253x; 1.2132x over previous
"""Trainium2 Bass kernel: multi-scale masked average-pool descriptors.

Computes, per batch element b and scribble i:
    d_l[b,i,c] = mean over {pixels where resize(scribble)[b,i,y,x] > 0.5} of feat_l[b,c,y,x]
    out[b,i,c] = (d_0 + d_1 + d_2) / 3

Strategy (v4 -- all-measured design):
  * jax.image.resize(bilinear, antialias=False) at scales 4/8/16 reduces to an
    exact 2x2 average at stride k with offset o (k,o) = (4,1)/(8,3)/(16,7):
    mask == ((a+c)+(b+d)) > 2.0 bit-exactly in fp32 (computed on DVE).
    Scribbles ride the gpsimd SWDGE queue as merged 4KB row-pair descriptors.
  * Feature maps are DMA'd with FULL-ROW descriptors ([y, c-group, x] tiles,
    one 512/256/128B descriptor per (c,y) row) on the two HWDGE rings -- the
    DMA descriptor walk performs the [c,y,x] -> [y,...] partition transpose
    for free (~233 GB/s measured; the xbar and PE transpose alternatives
    measured slower and/or serialize against all other DMA).
  * Engine copies re-pack each c-group [y, 32c, w] fp32 into assembled
    [y, x, 256c] bf16 tiles (cast during copy), so every matmul rhs is a
    contiguous [h, 256] bf16 slice.
  * ssum[i,:] accumulates as one matmul per pixel column x: lhsT =
    mask[:, :, x] [h, 16] bf16, rhs = f[:, x, :] [h, 256] bf16 -- measured
    251ns per LDWEIGHTS+MATMUL pair (strided rhs would be 779ns).
  * cnt[i] = reduce_sum over the mask + a ones-matmul; bf16 masks are exact
    0/1 and PSUM accumulates fp32, so cnt is exact and masks match the
    reference bit-exactly.  bf16 features give rel err ~2e-3 (gate: 2e-2).
  * The empty-mask fallback is handled on the host (P(empty) ~ 2^-1024).

Sharding: pure data-parallel over batch B=8 across the 8 NeuronCores.
"""

import numpy as np

_B = 8
_I = 16
_C = 256

# level: (h, k, off)
_LEVELS = {0: (128, 4, 1), 1: (64, 8, 3), 2: (32, 16, 7)}


def _build_nc():
    import concourse.bacc as bacc
    import concourse.tile as tile
    from concourse import mybir

    f32 = mybir.dt.float32
    bf16 = mybir.dt.bfloat16
    gt = mybir.AluOpType.is_gt
    X = mybir.AxisListType.X

    nc = bacc.Bacc("TRN2", target_bir_lowering=False, debug=False)

    feats = {
        0: nc.dram_tensor("feat0", [_C, 128, 128], f32, kind="ExternalInput"),
        1: nc.dram_tensor("feat1", [_C, 64, 64], f32, kind="ExternalInput"),
        2: nc.dram_tensor("feat2", [_C, 32, 32], f32, kind="ExternalInput"),
    }
    scr = nc.dram_tensor("scribbles", [_I, 512, 512], f32, kind="ExternalInput")
    out_d = nc.dram_tensor("out", [_I, 3 * (_C + 1)], f32, kind="ExternalOutput")

    with tile.TileContext(nc) as tc:
        with (
            tc.tile_pool(name="singles", bufs=1) as singles,
            tc.tile_pool(name="scrib", bufs=2) as scrib,
            tc.tile_pool(name="tmp", bufs=2) as tmp,
            tc.tile_pool(name="fR", bufs=2) as fR,
            tc.tile_pool(name="psum", bufs=3, space="PSUM") as psum,
        ):
            ones = singles.tile([128, 1], f32, tag="ones")
            nc.vector.memset(ones[:], 1.0)
            stag = singles.tile([_I, 3 * (_C + 1)], f32, tag="stag")

            # masks, y-on-partitions (natural resize layout): msk_l[y, i, x]
            msk0 = singles.tile([128, _I, 128], bf16, tag="msk0")
            msk1 = singles.tile([64, _I, 64], bf16, tag="msk1")
            msk2 = singles.tile([32, _I, 32], bf16, tag="msk2")
            msk = {0: msk0, 1: msk1, 2: msk2}
            # assembled feature tiles [y, x, c] bf16
            sgT0 = singles.tile([128, 128, _C], bf16, tag="sgT0")
            sgT1 = singles.tile([64, 64, _C], bf16, tag="sgT1")
            sgT2 = singles.tile([32, 32, _C], bf16, tag="sgT2")
            sgT = {0: sgT0, 1: sgT1, 2: sgT2}

            # ---- scribbles (gpsimd SWDGE; 4KB merged row-pair descriptors)
            # L0 first (it gates the longest matmul chain), pairs of images.
            for tpair in range(8):
                i0 = tpair * 2
                st = scrib.tile([128, 2, 1024], f32, tag="st0")
                nc.gpsimd.dma_start(
                    out=st[:],
                    in_=scr[i0 : i0 + 2]
                    .rearrange("i (y k) x -> y i k x", k=4)[:, :, 1:3, :]
                    .rearrange("y i k x -> y i (k x)"),
                )
                for il in range(2):
                    v = tmp.tile([128, 512], f32, tag="v")
                    nc.vector.tensor_add(
                        v[:], st[:, il, 0:512], st[:, il, 512:1024]
                    )
                    vk = v[:].rearrange("p (x k) -> p x k", k=4)
                    sr = tmp.tile([128, 128], f32, tag="sr")
                    nc.vector.tensor_add(sr[:], vk[:, :, 1], vk[:, :, 2])
                    nc.vector.tensor_scalar(
                        out=msk0[:, i0 + il, :], in0=sr[:], scalar1=2.0,
                        scalar2=None, op0=gt,
                    )
            # L1/L2 masks: the resize ALU runs on gpsimd so the DVE stays
            # free for the feature assembly copies (the critical path).
            for li, rr in ((1, 8), (2, 16)):
                h, k, off = _LEVELS[li]
                for i in range(_I):
                    st = scrib.tile([h, 2, 512], f32, tag=f"st{li}")
                    nc.gpsimd.dma_start(
                        out=st[:],
                        in_=scr[i].rearrange("(y r) x -> y r x", r=rr)[
                            :, off : off + 2, :
                        ],
                    )
                    v = tmp.tile([h, 512], f32, tag="vg")
                    nc.gpsimd.tensor_add(v[:], st[:, 0, :], st[:, 1, :])
                    vk = v[:].rearrange("p (x k) -> p x k", k=k)
                    sr = tmp.tile([h, h], f32, tag="srg")
                    nc.gpsimd.tensor_add(sr[:], vk[:, :, off], vk[:, :, off + 1])
                    nc.gpsimd.tensor_scalar(
                        out=msk[li][:, i, :], in0=sr[:], scalar1=2.0,
                        scalar2=None, op0=gt,
                    )

            # ---- features: full-row c-group loads + cast-assembly copies
            # (all copies on the DVE: measured 1.73ns/elem; scalar/gpsimd
            # copies are 2-4x slower)
            for li in (0, 1, 2):
                h = _LEVELS[li][0]
                for g in range(8):
                    sg = fR.tile([h, 32, h], f32, tag="sgR")
                    deng = nc.sync if g % 2 == 0 else nc.scalar
                    deng.dma_start(
                        out=sg[:],
                        in_=feats[li][32 * g : 32 * (g + 1)].rearrange(
                            "c y x -> y c x"
                        ),
                    )
                    nc.vector.tensor_copy(
                        sgT[li][:, :, 32 * g : 32 * (g + 1)],
                        sg[:].rearrange("p c x -> p x c"),
                    )

            # ---- matmuls + cnt + staging, level order 0, 1, 2
            for li in (0, 1, 2):
                h = _LEVELS[li][0]
                acc = psum.tile([_I, _C], f32, tag="acc")
                for x in range(h):
                    nc.tensor.matmul(
                        acc[:], msk[li][:, :, x], sgT[li][:, x, :],
                        start=(x == 0), stop=(x == h - 1),
                    )
                r = singles.tile([h, _I], f32, tag=f"r{li}")
                nc.vector.reduce_sum(out=r[:], in_=msk[li][:], axis=X)
                cnt = psum.tile([_I, 1], f32, tag="cnt")
                nc.tensor.matmul(cnt[:], r[:], ones[:h, :], start=True, stop=True)
                base = li * (_C + 1)
                nc.vector.tensor_copy(stag[:, base : base + _C], acc[:])
                nc.vector.tensor_copy(stag[:, base + _C : base + _C + 1], cnt[:])

            nc.gpsimd.dma_start(out=out_d[:], in_=stag[:])

    nc.compile()
    return nc


def _host_fallback(scr_bi, fmap_b, h, k, off):
    """Feature at argmax of the soft mask; only used when a mask is empty."""
    V = scr_bi[off::k, :][:h].astype(np.float32) + scr_bi[off + 1 :: k, :][:h]
    sr4 = V[:, off::k][:, :h] + V[:, off + 1 :: k][:, :h]
    idx = int(np.argmax(np.float32(0.25) * sr4))
    y, x = divmod(idx, h)
    return fmap_b[:, y, x]


def kernel(feat0, feat1, feat2, scribbles):
    import sys

    for p in ("/opt/trn_rl_repo", "/opt/pypackages"):
        if p not in sys.path:
            sys.path.append(p)
    from concourse.bass_utils import run_bass_kernel_spmd

    feat0 = np.asarray(feat0, dtype=np.float32)
    feat1 = np.asarray(feat1, dtype=np.float32)
    feat2 = np.asarray(feat2, dtype=np.float32)
    scribbles = np.asarray(scribbles, dtype=np.float32)

    nc = _build_nc()
    in_maps = [
        {
            "feat0": np.ascontiguousarray(feat0[b]),
            "feat1": np.ascontiguousarray(feat1[b]),
            "feat2": np.ascontiguousarray(feat2[b]),
            "scribbles": np.ascontiguousarray(scribbles[b]),
        }
        for b in range(_B)
    ]
    res = run_bass_kernel_spmd(nc, in_maps, core_ids=list(range(_B)))
    raw = np.stack([res.results[b]["out"] for b in range(_B)])  # [B, I, 3*257]
    raw = raw.reshape(_B, _I, 3, _C + 1)
    ssum = raw[..., :_C].astype(np.float32)  # [B, I, 3, C]
    cnt = raw[..., _C].astype(np.float32)  # [B, I, 3]

    mean = ssum / np.maximum(cnt, np.float32(1.0))[..., None]

    if (cnt == 0).any():  # never for non-degenerate inputs
        fm = [feat0, feat1, feat2]
        for b, i, li in zip(*np.nonzero(cnt == 0)):
            h, k, off = _LEVELS[li]
            mean[b, i, li] = _host_fallback(scribbles[b, i], fm[li][b], h, k, off)

    out = (mean[:, :, 0] + mean[:, :, 1] + mean[:, :, 2]) / np.float32(3.0)
    return out.astype(np.float32)


# revision 23
# speedup vs baseline: 1.8558x; 1.0756x over previous
"""Trainium2 Bass kernel: multi-scale masked average-pool descriptors.

Computes, per batch element b and scribble i:
    d_l[b,i,c] = mean over {pixels where resize(scribble)[b,i,y,x] > 0.5} of feat_l[b,c,y,x]
    out[b,i,c] = (d_0 + d_1 + d_2) / 3

Strategy (v4 -- all-measured design):
  * jax.image.resize(bilinear, antialias=False) at scales 4/8/16 reduces to an
    exact 2x2 average at stride k with offset o (k,o) = (4,1)/(8,3)/(16,7):
    mask == ((a+c)+(b+d)) > 2.0 bit-exactly in fp32 (computed on DVE).
    Scribbles ride the gpsimd SWDGE queue as merged 4KB row-pair descriptors.
  * Feature maps are DMA'd with FULL-ROW descriptors ([y, c-group, x] tiles,
    one 512/256/128B descriptor per (c,y) row) on the two HWDGE rings -- the
    DMA descriptor walk performs the [c,y,x] -> [y,...] partition transpose
    for free (~233 GB/s measured; the xbar and PE transpose alternatives
    measured slower and/or serialize against all other DMA).
  * Engine copies re-pack each c-group [y, 32c, w] fp32 into assembled
    [y, x, 256c] bf16 tiles (cast during copy), so every matmul rhs is a
    contiguous [h, 256] bf16 slice.
  * ssum[i,:] accumulates as one matmul per pixel column x: lhsT =
    mask[:, :, x] [h, 16] bf16, rhs = f[:, x, :] [h, 256] bf16 -- measured
    251ns per LDWEIGHTS+MATMUL pair (strided rhs would be 779ns).
  * cnt[i] = reduce_sum over the mask + a ones-matmul; bf16 masks are exact
    0/1 and PSUM accumulates fp32, so cnt is exact and masks match the
    reference bit-exactly.  bf16 features give rel err ~2e-3 (gate: 2e-2).
  * The empty-mask fallback is handled on the host (P(empty) ~ 2^-1024).

Sharding: pure data-parallel over batch B=8 across the 8 NeuronCores.
"""

import numpy as np

_B = 8
_I = 16
_C = 256

# level: (h, k, off)
_LEVELS = {0: (128, 4, 1), 1: (64, 8, 3), 2: (32, 16, 7)}


def _build_nc():
    import concourse.bacc as bacc
    import concourse.tile as tile
    from concourse import mybir

    f32 = mybir.dt.float32
    bf16 = mybir.dt.bfloat16
    gt = mybir.AluOpType.is_gt
    X = mybir.AxisListType.X

    nc = bacc.Bacc("TRN2", target_bir_lowering=False, debug=False)

    feats = {
        0: nc.dram_tensor("feat0", [_C, 128, 128], f32, kind="ExternalInput"),
        1: nc.dram_tensor("feat1", [_C, 64, 64], f32, kind="ExternalInput"),
        2: nc.dram_tensor("feat2", [_C, 32, 32], f32, kind="ExternalInput"),
    }
    scr = nc.dram_tensor("scribbles", [_I, 512, 512], f32, kind="ExternalInput")
    out_d = nc.dram_tensor("out", [_I, 3 * (_C + 1)], f32, kind="ExternalOutput")

    with tile.TileContext(nc) as tc:
        with (
            tc.tile_pool(name="singles", bufs=1) as singles,
            tc.tile_pool(name="scrib", bufs=2) as scrib,
            tc.tile_pool(name="tmp", bufs=2) as tmp,
            tc.tile_pool(name="fR", bufs=2) as fR,
            tc.tile_pool(name="psum", bufs=3, space="PSUM") as psum,
        ):
            ones = singles.tile([128, 1], f32, tag="ones")
            nc.vector.memset(ones[:], 1.0)
            stag = singles.tile([_I, 3 * (_C + 1)], f32, tag="stag")

            # masks, y-on-partitions (natural resize layout): msk_l[y, i, x]
            msk0 = singles.tile([128, _I, 128], bf16, tag="msk0")
            msk1 = singles.tile([64, _I, 64], bf16, tag="msk1")
            msk2 = singles.tile([32, _I, 32], bf16, tag="msk2")
            msk = {0: msk0, 1: msk1, 2: msk2}
            # assembled feature tiles [y, x, c] bf16
            sgT0 = singles.tile([128, 128, _C], bf16, tag="sgT0")
            sgT1 = singles.tile([64, 64, _C], bf16, tag="sgT1")
            sgT2 = singles.tile([32, 32, _C], bf16, tag="sgT2")
            sgT = {0: sgT0, 1: sgT1, 2: sgT2}

            # ---- interleaved per-level streams ----------------------
            # Queues: gpsimd = scribbles (4KB row-pair descs), sync/scalar =
            # feature full-row loads.  The DVE FIFO alternates one feature
            # assembly copy with one tile's mask ALU so neither stream
            # stalls the other; emission order == engine FIFO order.

            def mask_ops(li, i, st, il=None):
                h, k, off = _LEVELS[li]
                src_lo = st[:, il, 0:512] if il is not None else st[:, 0, :]
                src_hi = st[:, il, 512:1024] if il is not None else st[:, 1, :]
                v = tmp.tile([h, 512], f32, tag="v")
                nc.vector.tensor_add(v[:], src_lo, src_hi)
                vk = v[:].rearrange("p (x k) -> p x k", k=k)
                sr = tmp.tile([h, h], f32, tag="sr")
                nc.vector.tensor_add(sr[:], vk[:, :, off], vk[:, :, off + 1])
                nc.vector.tensor_scalar(
                    out=msk[li][:, i, :], in0=sr[:], scalar1=2.0,
                    scalar2=None, op0=gt,
                )

            def feat_load_copy(li, g):
                h = _LEVELS[li][0]
                sg = fR.tile([h, 32, h], f32, tag="sgR")
                deng = nc.sync if g % 2 == 0 else nc.scalar
                deng.dma_start(
                    out=sg[:],
                    in_=feats[li][32 * g : 32 * (g + 1)].rearrange(
                        "c y x -> y c x"
                    ),
                )
                nc.vector.tensor_copy(
                    sgT[li][:, :, 32 * g : 32 * (g + 1)],
                    sg[:].rearrange("p c x -> p x c"),
                )

            # L0: 8 iterations of {scribble pair, feature group, copy, masks}
            for t in range(8):
                i0 = t * 2
                st = scrib.tile([128, 2, 1024], f32, tag="st0")
                nc.gpsimd.dma_start(
                    out=st[:],
                    in_=scr[i0 : i0 + 2]
                    .rearrange("i (y k) x -> y i k x", k=4)[:, :, 1:3, :]
                    .rearrange("y i k x -> y i (k x)"),
                )
                feat_load_copy(0, t)
                for il in range(2):
                    mask_ops(0, i0 + il, st, il=il)

            # L1/L2: 16 iterations of {scribble, (feature group + copy)/2, mask}
            for li, rr in ((1, 8), (2, 16)):
                h, k, off = _LEVELS[li]
                for i in range(_I):
                    st = scrib.tile([h, 2, 512], f32, tag=f"st{li}")
                    nc.gpsimd.dma_start(
                        out=st[:],
                        in_=scr[i].rearrange("(y r) x -> y r x", r=rr)[
                            :, off : off + 2, :
                        ],
                    )
                    if i % 2 == 0:
                        feat_load_copy(li, i // 2)
                    mask_ops(li, i, st)

            # ---- matmuls + cnt + staging, level order 0, 1, 2
            for li in (0, 1, 2):
                h = _LEVELS[li][0]
                acc = psum.tile([_I, _C], f32, tag="acc")
                for x in range(h):
                    nc.tensor.matmul(
                        acc[:], msk[li][:, :, x], sgT[li][:, x, :],
                        start=(x == 0), stop=(x == h - 1),
                    )
                r = singles.tile([h, _I], f32, tag=f"r{li}")
                nc.vector.reduce_sum(out=r[:], in_=msk[li][:], axis=X)
                cnt = psum.tile([_I, 1], f32, tag="cnt")
                nc.tensor.matmul(cnt[:], r[:], ones[:h, :], start=True, stop=True)
                base = li * (_C + 1)
                nc.vector.tensor_copy(stag[:, base : base + _C], acc[:])
                nc.vector.tensor_copy(stag[:, base + _C : base + _C + 1], cnt[:])

            nc.gpsimd.dma_start(out=out_d[:], in_=stag[:])

    nc.compile()
    return nc


def _host_fallback(scr_bi, fmap_b, h, k, off):
    """Feature at argmax of the soft mask; only used when a mask is empty."""
    V = scr_bi[off::k, :][:h].astype(np.float32) + scr_bi[off + 1 :: k, :][:h]
    sr4 = V[:, off::k][:, :h] + V[:, off + 1 :: k][:, :h]
    idx = int(np.argmax(np.float32(0.25) * sr4))
    y, x = divmod(idx, h)
    return fmap_b[:, y, x]


def kernel(feat0, feat1, feat2, scribbles):
    import sys

    for p in ("/opt/trn_rl_repo", "/opt/pypackages"):
        if p not in sys.path:
            sys.path.append(p)
    from concourse.bass_utils import run_bass_kernel_spmd

    feat0 = np.asarray(feat0, dtype=np.float32)
    feat1 = np.asarray(feat1, dtype=np.float32)
    feat2 = np.asarray(feat2, dtype=np.float32)
    scribbles = np.asarray(scribbles, dtype=np.float32)

    nc = _build_nc()
    in_maps = [
        {
            "feat0": np.ascontiguousarray(feat0[b]),
            "feat1": np.ascontiguousarray(feat1[b]),
            "feat2": np.ascontiguousarray(feat2[b]),
            "scribbles": np.ascontiguousarray(scribbles[b]),
        }
        for b in range(_B)
    ]
    res = run_bass_kernel_spmd(nc, in_maps, core_ids=list(range(_B)))
    raw = np.stack([res.results[b]["out"] for b in range(_B)])  # [B, I, 3*257]
    raw = raw.reshape(_B, _I, 3, _C + 1)
    ssum = raw[..., :_C].astype(np.float32)  # [B, I, 3, C]
    cnt = raw[..., _C].astype(np.float32)  # [B, I, 3]

    mean = ssum / np.maximum(cnt, np.float32(1.0))[..., None]

    if (cnt == 0).any():  # never for non-degenerate inputs
        fm = [feat0, feat1, feat2]
        for b, i, li in zip(*np.nonzero(cnt == 0)):
            h, k, off = _LEVELS[li]
            mean[b, i, li] = _host_fallback(scribbles[b, i], fm[li][b], h, k, off)

    out = (mean[:, :, 0] + mean[:, :, 1] + mean[:, :, 2]) / np.float32(3.0)
    return out.astype(np.float32)


# revision 24
# speedup vs baseline: 2.0011x; 1.0783x over previous
"""Trainium2 Bass kernel: multi-scale masked average-pool descriptors.

Computes, per batch element b and scribble i:
    d_l[b,i,c] = mean over {pixels where resize(scribble)[b,i,y,x] > 0.5} of feat_l[b,c,y,x]
    out[b,i,c] = (d_0 + d_1 + d_2) / 3

Strategy (v4 -- all-measured design):
  * jax.image.resize(bilinear, antialias=False) at scales 4/8/16 reduces to an
    exact 2x2 average at stride k with offset o (k,o) = (4,1)/(8,3)/(16,7):
    mask == ((a+c)+(b+d)) > 2.0 bit-exactly in fp32 (computed on DVE).
    Scribbles ride the gpsimd SWDGE queue as merged 4KB row-pair descriptors.
  * Feature maps are DMA'd with FULL-ROW descriptors ([y, c-group, x] tiles,
    one 512/256/128B descriptor per (c,y) row) on the two HWDGE rings -- the
    DMA descriptor walk performs the [c,y,x] -> [y,...] partition transpose
    for free (~233 GB/s measured; the xbar and PE transpose alternatives
    measured slower and/or serialize against all other DMA).
  * Engine copies re-pack each c-group [y, 32c, w] fp32 into assembled
    [y, x, 256c] bf16 tiles (cast during copy), so every matmul rhs is a
    contiguous [h, 256] bf16 slice.
  * ssum[i,:] accumulates as one matmul per pixel column x: lhsT =
    mask[:, :, x] [h, 16] bf16, rhs = f[:, x, :] [h, 256] bf16 -- measured
    251ns per LDWEIGHTS+MATMUL pair (strided rhs would be 779ns).
  * cnt[i] = reduce_sum over the mask + a ones-matmul; bf16 masks are exact
    0/1 and PSUM accumulates fp32, so cnt is exact and masks match the
    reference bit-exactly.  bf16 features give rel err ~2e-3 (gate: 2e-2).
  * The empty-mask fallback is handled on the host (P(empty) ~ 2^-1024).

Sharding: pure data-parallel over batch B=8 across the 8 NeuronCores.
"""

import numpy as np

_B = 8
_I = 16
_C = 256

# level: (h, k, off)
_LEVELS = {0: (128, 4, 1), 1: (64, 8, 3), 2: (32, 16, 7)}


def _build_nc():
    import concourse.bacc as bacc
    import concourse.tile as tile
    from concourse import mybir

    f32 = mybir.dt.float32
    bf16 = mybir.dt.bfloat16
    gt = mybir.AluOpType.is_gt
    X = mybir.AxisListType.X

    nc = bacc.Bacc("TRN2", target_bir_lowering=False, debug=False)

    feats = {
        0: nc.dram_tensor("feat0", [_C, 128, 128], f32, kind="ExternalInput"),
        1: nc.dram_tensor("feat1", [_C, 64, 64], f32, kind="ExternalInput"),
        2: nc.dram_tensor("feat2", [_C, 32, 32], f32, kind="ExternalInput"),
    }
    scr = nc.dram_tensor("scribbles", [_I, 512, 512], f32, kind="ExternalInput")
    out_d = nc.dram_tensor("out", [_I, 3 * (_C + 1)], f32, kind="ExternalOutput")

    with tile.TileContext(nc) as tc:
        with (
            tc.tile_pool(name="singles", bufs=1) as singles,
            tc.tile_pool(name="scrib", bufs=2) as scrib,
            tc.tile_pool(name="tmp", bufs=2) as tmp,
            tc.tile_pool(name="fR", bufs=4) as fR,
            tc.tile_pool(name="psum", bufs=3, space="PSUM") as psum,
        ):
            ones = singles.tile([128, 1], f32, tag="ones")
            nc.vector.memset(ones[:], 1.0)
            stag = singles.tile([_I, 3 * (_C + 1)], f32, tag="stag")

            # masks, y-on-partitions (natural resize layout): msk_l[y, i, x]
            msk0 = singles.tile([128, _I, 128], bf16, tag="msk0")
            msk1 = singles.tile([64, _I, 64], bf16, tag="msk1")
            msk2 = singles.tile([32, _I, 32], bf16, tag="msk2")
            msk = {0: msk0, 1: msk1, 2: msk2}
            # assembled feature tiles [y, x, c] bf16
            sgT0 = singles.tile([128, 128, _C], bf16, tag="sgT0")
            sgT1 = singles.tile([64, 64, _C], bf16, tag="sgT1")
            sgT2 = singles.tile([32, 32, _C], bf16, tag="sgT2")
            sgT = {0: sgT0, 1: sgT1, 2: sgT2}

            # ---- interleaved per-level streams ----------------------
            # Queues: gpsimd = scribbles (4KB row-pair descs), sync/scalar =
            # feature full-row loads.  The DVE FIFO alternates one feature
            # assembly copy with one tile's mask ALU so neither stream
            # stalls the other; emission order == engine FIFO order.

            def mask_ops(li, i, st, il=None):
                # only the 2-of-k needed resize columns are added (strided)
                h, k, off = _LEVELS[li]
                src_lo = st[:, il, 0:512] if il is not None else st[:, 0, :]
                src_hi = st[:, il, 512:1024] if il is not None else st[:, 1, :]
                a = src_lo.rearrange("p (x k) -> p x k", k=k)[:, :, off : off + 2]
                b = src_hi.rearrange("p (x k) -> p x k", k=k)[:, :, off : off + 2]
                v = tmp.tile([h, h, 2], f32, tag="v")
                nc.vector.tensor_add(v[:], a, b)
                sr = tmp.tile([h, h], f32, tag="sr")
                nc.vector.tensor_add(sr[:], v[:, :, 0], v[:, :, 1])
                nc.vector.tensor_scalar(
                    out=msk[li][:, i, :], in0=sr[:], scalar1=2.0,
                    scalar2=None, op0=gt,
                )

            def feat_load_copy(li, g):
                h = _LEVELS[li][0]
                sg = fR.tile([h, 16, h], f32, tag="sgR")
                deng = nc.sync if g % 2 == 0 else nc.scalar
                deng.dma_start(
                    out=sg[:],
                    in_=feats[li][16 * g : 16 * (g + 1)].rearrange(
                        "c y x -> y c x"
                    ),
                )
                nc.vector.tensor_copy(
                    sgT[li][:, :, 16 * g : 16 * (g + 1)],
                    sg[:].rearrange("p c x -> p x c"),
                )

            # L0: 8 iterations of {scribble pair, feature group, copy, masks}
            for t in range(8):
                i0 = t * 2
                st = scrib.tile([128, 2, 1024], f32, tag="st0")
                nc.gpsimd.dma_start(
                    out=st[:],
                    in_=scr[i0 : i0 + 2]
                    .rearrange("i (y k) x -> y i k x", k=4)[:, :, 1:3, :]
                    .rearrange("y i k x -> y i (k x)"),
                )
                for il in range(2):
                    feat_load_copy(0, 2 * t + il)
                    mask_ops(0, i0 + il, st, il=il)

            # L1/L2: 16 iterations of {scribble, (feature group + copy)/2, mask}
            for li, rr in ((1, 8), (2, 16)):
                h, k, off = _LEVELS[li]
                for i in range(_I):
                    st = scrib.tile([h, 2, 512], f32, tag=f"st{li}")
                    nc.gpsimd.dma_start(
                        out=st[:],
                        in_=scr[i].rearrange("(y r) x -> y r x", r=rr)[
                            :, off : off + 2, :
                        ],
                    )
                    feat_load_copy(li, i)
                    mask_ops(li, i, st)

            # ---- matmuls + cnt + staging, level order 0, 1, 2
            for li in (0, 1, 2):
                h = _LEVELS[li][0]
                acc = psum.tile([_I, _C], f32, tag="acc")
                for x in range(h):
                    nc.tensor.matmul(
                        acc[:], msk[li][:, :, x], sgT[li][:, x, :],
                        start=(x == 0), stop=(x == h - 1),
                    )
                r = singles.tile([h, _I], f32, tag=f"r{li}")
                nc.vector.reduce_sum(out=r[:], in_=msk[li][:], axis=X)
                cnt = psum.tile([_I, 1], f32, tag="cnt")
                nc.tensor.matmul(cnt[:], r[:], ones[:h, :], start=True, stop=True)
                base = li * (_C + 1)
                nc.vector.tensor_copy(stag[:, base : base + _C], acc[:])
                nc.vector.tensor_copy(stag[:, base + _C : base + _C + 1], cnt[:])

            nc.gpsimd.dma_start(out=out_d[:], in_=stag[:])

    nc.compile()
    return nc


def _host_fallback(scr_bi, fmap_b, h, k, off):
    """Feature at argmax of the soft mask; only used when a mask is empty."""
    V = scr_bi[off::k, :][:h].astype(np.float32) + scr_bi[off + 1 :: k, :][:h]
    sr4 = V[:, off::k][:, :h] + V[:, off + 1 :: k][:, :h]
    idx = int(np.argmax(np.float32(0.25) * sr4))
    y, x = divmod(idx, h)
    return fmap_b[:, y, x]


def kernel(feat0, feat1, feat2, scribbles):
    import sys

    for p in ("/opt/trn_rl_repo", "/opt/pypackages"):
        if p not in sys.path:
            sys.path.append(p)
    from concourse.bass_utils import run_bass_kernel_spmd

    feat0 = np.asarray(feat0, dtype=np.float32)
    feat1 = np.asarray(feat1, dtype=np.float32)
    feat2 = np.asarray(feat2, dtype=np.float32)
    scribbles = np.asarray(scribbles, dtype=np.float32)

    nc = _build_nc()
    in_maps = [
        {
            "feat0": np.ascontiguousarray(feat0[b]),
            "feat1": np.ascontiguousarray(feat1[b]),
            "feat2": np.ascontiguousarray(feat2[b]),
            "scribbles": np.ascontiguousarray(scribbles[b]),
        }
        for b in range(_B)
    ]
    res = run_bass_kernel_spmd(nc, in_maps, core_ids=list(range(_B)))
    raw = np.stack([res.results[b]["out"] for b in range(_B)])  # [B, I, 3*257]
    raw = raw.reshape(_B, _I, 3, _C + 1)
    ssum = raw[..., :_C].astype(np.float32)  # [B, I, 3, C]
    cnt = raw[..., _C].astype(np.float32)  # [B, I, 3]

    mean = ssum / np.maximum(cnt, np.float32(1.0))[..., None]

    if (cnt == 0).any():  # never for non-degenerate inputs
        fm = [feat0, feat1, feat2]
        for b, i, li in zip(*np.nonzero(cnt == 0)):
            h, k, off = _LEVELS[li]
            mean[b, i, li] = _host_fallback(scribbles[b, i], fm[li][b], h, k, off)

    out = (mean[:, :, 0] + mean[:, :, 1] + mean[:, :, 2]) / np.float32(3.0)
    return out.astype(np.float32)


# revision 25
# speedup vs baseline: 2.0583x; 1.0286x over previous
"""Trainium2 Bass kernel: multi-scale masked average-pool descriptors.

Computes, per batch element b and scribble i:
    d_l[b,i,c] = mean over {pixels where resize(scribble)[b,i,y,x] > 0.5} of feat_l[b,c,y,x]
    out[b,i,c] = (d_0 + d_1 + d_2) / 3

Strategy (v4 -- all-measured design):
  * jax.image.resize(bilinear, antialias=False) at scales 4/8/16 reduces to an
    exact 2x2 average at stride k with offset o (k,o) = (4,1)/(8,3)/(16,7):
    mask == ((a+c)+(b+d)) > 2.0 bit-exactly in fp32 (computed on DVE).
    Scribbles ride the gpsimd SWDGE queue as merged 4KB row-pair descriptors.
  * Feature maps are DMA'd with FULL-ROW descriptors ([y, c-group, x] tiles,
    one 512/256/128B descriptor per (c,y) row) on the two HWDGE rings -- the
    DMA descriptor walk performs the [c,y,x] -> [y,...] partition transpose
    for free (~233 GB/s measured; the xbar and PE transpose alternatives
    measured slower and/or serialize against all other DMA).
  * Engine copies re-pack each c-group [y, 32c, w] fp32 into assembled
    [y, x, 256c] bf16 tiles (cast during copy), so every matmul rhs is a
    contiguous [h, 256] bf16 slice.
  * ssum[i,:] accumulates as one matmul per pixel column x: lhsT =
    mask[:, :, x] [h, 16] bf16, rhs = f[:, x, :] [h, 256] bf16 -- measured
    251ns per LDWEIGHTS+MATMUL pair (strided rhs would be 779ns).
  * cnt[i] = reduce_sum over the mask + a ones-matmul; bf16 masks are exact
    0/1 and PSUM accumulates fp32, so cnt is exact and masks match the
    reference bit-exactly.  bf16 features give rel err ~2e-3 (gate: 2e-2).
  * The empty-mask fallback is handled on the host (P(empty) ~ 2^-1024).

Sharding: pure data-parallel over batch B=8 across the 8 NeuronCores.
"""

import numpy as np

_B = 8
_I = 16
_C = 256

# level: (h, k, off)
_LEVELS = {0: (128, 4, 1), 1: (64, 8, 3), 2: (32, 16, 7)}


def _build_nc():
    import concourse.bacc as bacc
    import concourse.tile as tile
    from concourse import mybir

    f32 = mybir.dt.float32
    bf16 = mybir.dt.bfloat16
    gt = mybir.AluOpType.is_gt
    X = mybir.AxisListType.X

    nc = bacc.Bacc("TRN2", target_bir_lowering=False, debug=False)

    feats = {
        0: nc.dram_tensor("feat0", [_C, 128, 128], f32, kind="ExternalInput"),
        1: nc.dram_tensor("feat1", [_C, 64, 64], f32, kind="ExternalInput"),
        2: nc.dram_tensor("feat2", [_C, 32, 32], f32, kind="ExternalInput"),
    }
    scr = nc.dram_tensor("scribbles", [_I, 512, 512], f32, kind="ExternalInput")
    out_d = nc.dram_tensor("out", [_I, 3 * (_C + 1)], f32, kind="ExternalOutput")

    with tile.TileContext(nc) as tc:
        with (
            tc.tile_pool(name="singles", bufs=1) as singles,
            tc.tile_pool(name="scrib", bufs=2) as scrib,
            tc.tile_pool(name="tmp", bufs=2) as tmp,
            tc.tile_pool(name="fR", bufs=4) as fR,
            tc.tile_pool(name="psum", bufs=3, space="PSUM") as psum,
        ):
            ones = singles.tile([128, 1], f32, tag="ones")
            nc.vector.memset(ones[:], 1.0)
            stag = singles.tile([_I, 3 * (_C + 1)], f32, tag="stag")

            # masks, y-on-partitions (natural resize layout): msk_l[y, i, x]
            msk0 = singles.tile([128, _I, 128], bf16, tag="msk0")
            msk1 = singles.tile([64, _I, 64], bf16, tag="msk1")
            msk2 = singles.tile([32, _I, 32], bf16, tag="msk2")
            msk = {0: msk0, 1: msk1, 2: msk2}
            # assembled feature tiles [y, x, c] bf16
            sgT0 = singles.tile([128, 128, _C], bf16, tag="sgT0")
            sgT1 = singles.tile([64, 64, _C], bf16, tag="sgT1")
            sgT2 = singles.tile([32, 32, _C], bf16, tag="sgT2")
            sgT = {0: sgT0, 1: sgT1, 2: sgT2}

            # ---- interleaved per-level streams ----------------------
            # Queues: gpsimd = scribbles (4KB row-pair descs), sync/scalar =
            # feature full-row loads.  The DVE FIFO alternates one feature
            # assembly copy with one tile's mask ALU so neither stream
            # stalls the other; emission order == engine FIFO order.

            def mask_ops(li, i, st, il=None):
                # only the 2-of-k needed resize columns are added (strided)
                h, k, off = _LEVELS[li]
                src_lo = st[:, il, 0:512] if il is not None else st[:, 0, :]
                src_hi = st[:, il, 512:1024] if il is not None else st[:, 1, :]
                a = src_lo.rearrange("p (x k) -> p x k", k=k)[:, :, off : off + 2]
                b = src_hi.rearrange("p (x k) -> p x k", k=k)[:, :, off : off + 2]
                v = tmp.tile([h, h, 2], f32, tag="v")
                nc.vector.tensor_add(v[:], a, b)
                sr = tmp.tile([h, h], f32, tag="sr")
                nc.vector.tensor_add(sr[:], v[:, :, 0], v[:, :, 1])
                nc.vector.tensor_scalar(
                    out=msk[li][:, i, :], in0=sr[:], scalar1=2.0,
                    scalar2=None, op0=gt,
                )

            def feat_load_copy(li, g):
                h = _LEVELS[li][0]
                sg = fR.tile([h, 16, h], f32, tag="sgR")
                deng = nc.sync if g % 2 == 0 else nc.scalar
                deng.dma_start(
                    out=sg[:],
                    in_=feats[li][16 * g : 16 * (g + 1)].rearrange(
                        "c y x -> y c x"
                    ),
                )
                if g % 2 == 0:
                    nc.vector.tensor_copy(
                        sgT[li][:, :, 16 * g : 16 * (g + 1)],
                        sg[:].rearrange("p c x -> p x c"),
                    )
                else:
                    nc.scalar.copy(
                        sgT[li][:, :, 16 * g : 16 * (g + 1)],
                        sg[:].rearrange("p c x -> p x c"),
                    )

            # L0: 8 iterations of {scribble pair, feature group, copy, masks}
            for t in range(8):
                i0 = t * 2
                st = scrib.tile([128, 2, 1024], f32, tag="st0")
                nc.gpsimd.dma_start(
                    out=st[:],
                    in_=scr[i0 : i0 + 2]
                    .rearrange("i (y k) x -> y i k x", k=4)[:, :, 1:3, :]
                    .rearrange("y i k x -> y i (k x)"),
                )
                for il in range(2):
                    feat_load_copy(0, 2 * t + il)
                    mask_ops(0, i0 + il, st, il=il)

            # L1/L2: 16 iterations of {scribble, (feature group + copy)/2, mask}
            for li, rr in ((1, 8), (2, 16)):
                h, k, off = _LEVELS[li]
                for i in range(_I):
                    st = scrib.tile([h, 2, 512], f32, tag=f"st{li}")
                    nc.gpsimd.dma_start(
                        out=st[:],
                        in_=scr[i].rearrange("(y r) x -> y r x", r=rr)[
                            :, off : off + 2, :
                        ],
                    )
                    feat_load_copy(li, i)
                    mask_ops(li, i, st)

            # ---- matmuls + cnt + staging, level order 0, 1, 2
            for li in (0, 1, 2):
                h = _LEVELS[li][0]
                acc = psum.tile([_I, _C], f32, tag="acc")
                for x in range(h):
                    nc.tensor.matmul(
                        acc[:], msk[li][:, :, x], sgT[li][:, x, :],
                        start=(x == 0), stop=(x == h - 1),
                    )
                r = singles.tile([h, _I], f32, tag=f"r{li}")
                nc.vector.reduce_sum(out=r[:], in_=msk[li][:], axis=X)
                cnt = psum.tile([_I, 1], f32, tag="cnt")
                nc.tensor.matmul(cnt[:], r[:], ones[:h, :], start=True, stop=True)
                base = li * (_C + 1)
                nc.vector.tensor_copy(stag[:, base : base + _C], acc[:])
                nc.vector.tensor_copy(stag[:, base + _C : base + _C + 1], cnt[:])

            nc.gpsimd.dma_start(out=out_d[:], in_=stag[:])

    nc.compile()
    return nc


def _host_fallback(scr_bi, fmap_b, h, k, off):
    """Feature at argmax of the soft mask; only used when a mask is empty."""
    V = scr_bi[off::k, :][:h].astype(np.float32) + scr_bi[off + 1 :: k, :][:h]
    sr4 = V[:, off::k][:, :h] + V[:, off + 1 :: k][:, :h]
    idx = int(np.argmax(np.float32(0.25) * sr4))
    y, x = divmod(idx, h)
    return fmap_b[:, y, x]


def kernel(feat0, feat1, feat2, scribbles):
    import sys

    for p in ("/opt/trn_rl_repo", "/opt/pypackages"):
        if p not in sys.path:
            sys.path.append(p)
    from concourse.bass_utils import run_bass_kernel_spmd

    feat0 = np.asarray(feat0, dtype=np.float32)
    feat1 = np.asarray(feat1, dtype=np.float32)
    feat2 = np.asarray(feat2, dtype=np.float32)
    scribbles = np.asarray(scribbles, dtype=np.float32)

    nc = _build_nc()
    in_maps = [
        {
            "feat0": np.ascontiguousarray(feat0[b]),
            "feat1": np.ascontiguousarray(feat1[b]),
            "feat2": np.ascontiguousarray(feat2[b]),
            "scribbles": np.ascontiguousarray(scribbles[b]),
        }
        for b in range(_B)
    ]
    res = run_bass_kernel_spmd(nc, in_maps, core_ids=list(range(_B)))
    raw = np.stack([res.results[b]["out"] for b in range(_B)])  # [B, I, 3*257]
    raw = raw.reshape(_B, _I, 3, _C + 1)
    ssum = raw[..., :_C].astype(np.float32)  # [B, I, 3, C]
    cnt = raw[..., _C].astype(np.float32)  # [B, I, 3]

    mean = ssum / np.maximum(cnt, np.float32(1.0))[..., None]

    if (cnt == 0).any():  # never for non-degenerate inputs
        fm = [feat0, feat1, feat2]
        for b, i, li in zip(*np.nonzero(cnt == 0)):
            h, k, off = _LEVELS[li]
            mean[b, i, li] = _host_fallback(scribbles[b, i], fm[li][b], h, k, off)

    out = (mean[:, :, 0] + mean[:, :, 1] + mean[:, :, 2]) / np.float32(3.0)
    return out.astype(np.float32)
